# revision 4
# baseline (speedup 1.0000x reference)
# Trainium2 Bass kernel for the DVAE encoder (nn_DVAE_24850680775463).
#
# Sharding: pure data-parallel. B=1024 graphs -> 8 cores x 128 graphs.
# Per core, the 128 graphs sit on the 128 SBUF partitions and the whole
# 16-vertex sequential scan runs on-chip.
#
# Structure (v2 — half-tile pipelined):
#  * gate/mapper products computed once per vertex (incremental cache);
#    vertex-id one-hot contribution folds into a per-vertex bias row.
#  * message H_v = sum_n adj[b,v,n] * gm[b,n,:] via PSUM-accumulated
#    matmuls with diag(adj[:,v,n]) stationary.
#  * GRU x-side + biases enter via small matmuls accumulating into the
#    same PSUM banks as the h-side matmuls.
#  * All elementwise / activation work is split into 256-wide halves so
#    the serial chain (sigma -> mul -> add -> tanh -> ...) pipelines across
#    the Scalar/Vector/GpSimd engines at half-tile granularity.
#  * z-path (1-z, z*h) runs early on GpSimd as soon as sigma(Z) lands, so
#    only zn/hv remain after tanh on the critical path.
#  * diag builds for messages run on GpSimd; transposes chunk-pipeline
#    with their PSUM->SBUF copies.
#  * small dependency-free PE fills keep the Tensor engine's p-state at
#    full clock through the elementwise windows.
# Matmuls default to bf16 (fp32 PSUM accumulation).

import os
import numpy as np

import concourse.bass as bass
import concourse.tile as tile
from concourse import bacc, mybir
from concourse.bass_utils import run_bass_kernel_spmd

AF = mybir.ActivationFunctionType
F32 = mybir.dt.float32

NCORES = 8
B, NV, NVT, FS, HS, NZ = 1024, 16, 16, 32, 512, 64
P = B // NCORES            # 128 graphs per core
G3 = 3 * HS                # 1536
K1 = NVT + 1               # 17  (one-hot + ones row)
K2 = FS + 1                # 33  (params + ones row)
KC = HS // 128             # 4 contraction chunks of the hidden dim
HF = HS // 2               # 256 half width

MMDT = {"f32r": mybir.dt.float32r, "f32": mybir.dt.float32,
        "bf16": mybir.dt.bfloat16}[os.environ.get("DVAE_MMDT", "bf16")]


def build_bass():
    nc = bacc.Bacc("TRN2", target_bir_lowering=False, debug=False)

    def inp(name, shape, dt=None):
        return nc.dram_tensor(name, shape, dt or MMDT,
                              kind="ExternalInput").ap()

    d = {
        "wht_t": inp("wht_t", [128, KC * G3]),
        "wht_p": inp("wht_p", [128, KC * G3]),
        "w1x":   inp("w1x",   [K1, G3]),
        "w2x":   inp("w2x",   [K2, G3]),
        "xt1":   inp("xt1",   [K1, NV * P]),
        "xp1":   inp("xp1",   [K2, NV * P]),
        "bhn2":  inp("bhn2",  [1, 2 * HS]),
        "wgm":   inp("wgm",   [128, KC * 2 * HS]),
        "bgm":   inp("bgm",   [NV, 2 * HS]),
        "vsel":  inp("vsel",  [NV, NV * P]),
        "adjt":  inp("adjt",  [P, NV * NV], F32),
        "wfc":   inp("wfc",   [128, KC * 2 * NZ]),
        "bfc":   inp("bfc",   [1, 2 * NZ]),
        "eye":   inp("eye",   [128, 128], F32),
        "eyeb":  inp("eyeb",  [128, 128]),
        "ones1": inp("ones1", [1, 128]),
    }
    out_ap = nc.dram_tensor("out", [P, 2 * NZ], mybir.dt.float32, kind="ExternalOutput").ap()

    with tile.TileContext(nc) as tc:
        _body(tc, d, out_ap)
    nc.compile()
    return nc


def _body(tc, d, out_ap):
    nc = tc.nc
    from contextlib import ExitStack
    with ExitStack() as ctx:
        wp = ctx.enter_context(tc.tile_pool(name="w", bufs=1))
        sp = ctx.enter_context(tc.tile_pool(name="s", bufs=1))
        dgp = ctx.enter_context(tc.tile_pool(name="dg", bufs=16))
        gmc = ctx.enter_context(tc.tile_pool(name="gmc", bufs=1))
        ps_h = ctx.enter_context(tc.tile_pool(name="psh", bufs=1, space="PSUM"))
        ps_g = ctx.enter_context(tc.tile_pool(name="psg", bufs=4, space="PSUM"))
        ps_t = ctx.enter_context(tc.tile_pool(name="pst", bufs=1, space="PSUM"))
        ps_m = ctx.enter_context(tc.tile_pool(name="psm", bufs=1, space="PSUM"))

        # ---- persistent weights / constants ----
        order = ["ones1", "w1x", "xt1", "bhn2", "eye", "eyeb", "adjt", "xp1",
                 "w2x", "wgm", "wht_p", "vsel", "bgm", "wht_t", "wfc", "bfc"]
        W = {}
        for name, ap in sorted(d.items(), key=lambda kv: order.index(kv[0])):
            t = wp.tile(list(ap.shape), ap.dtype, tag=name)
            nc.sync.dma_start(t[:], ap[:, :])
            W[name] = t

        wht = {0: W["wht_t"], 1: W["wht_p"]}
        wx = {0: W["w1x"], 1: W["w2x"]}
        xs = {0: W["xt1"], 1: W["xp1"]}
        kx = {0: K1, 1: K2}
        eye = W["eye"]
        adjt = W["adjt"]

        gm_sb = []          # cached gate*mapped per vertex, [P, HS] each

        def halves(t):
            return (t[:, 0:HF], t[:, HF:HS])

        def transpose_block(src_ab, tag, copy_eng):
            """src_ab: two [128,256] bf16 batch-major halves (or one [128,512]
            tile's halves). PE-transpose chunk-by-chunk; copies pipelined.
            copy_eng: 'v' or 's' for the PSUM->SBUF chunk copies."""
            tp = ps_t.tile([128, HS], MMDT, tag="tp")
            dst = sp.tile([128, HS], MMDT, tag=tag)
            for c in range(KC):
                src = src_ab[c // 2]
                nc.tensor.transpose(tp[:, c * 128:(c + 1) * 128],
                                    src[:, (c % 2) * 128:(c % 2 + 1) * 128],
                                    W["eyeb"][:])
            for h in range(2):
                sl = slice(h * HF, (h + 1) * HF)
                if copy_eng == 'v':
                    nc.vector.tensor_copy(dst[:, sl], tp[:, sl])
                else:
                    nc.scalar.copy(dst[:, sl], tp[:, sl])
            return dst

        def gru_mms(g, v, HT):
            """Emit all matmuls of GRU g (0=type,1=param) for vertex v.
            HT: feature-major hidden state [128, 512] or None (h=0).
            Gate emission order: R first (its sigma leads the chain), then
            NH (rhn needs it 2nd), NI, Z."""
            K = kx[g]
            xl = xs[g][:K, v * P:(v + 1) * P]      # lhsT [K,128]
            wxr = wx[g]
            w = wht[g]
            R = ps_g.tile([128, HS], F32, tag="g")
            NH = ps_g.tile([128, HS], F32, tag="g")
            NI = ps_g.tile([128, HS], F32, tag="g")
            Z = ps_g.tile([128, HS], F32, tag="g")
            nc.tensor.matmul(R[:], xl, wxr[:K, 0:HS],
                             start=True, stop=HT is None)
            if HT is not None:
                for c in range(KC):
                    nc.tensor.matmul(R[:], HT[:, c * 128:(c + 1) * 128],
                                     w[:, c * G3:c * G3 + HS],
                                     start=False, stop=c == KC - 1)
            nc.tensor.matmul(NH[:], W["ones1"][:],
                             W["bhn2"][0:1, g * HS:(g + 1) * HS],
                             start=True, stop=HT is None)
            if HT is not None:
                for c in range(KC):
                    nc.tensor.matmul(NH[:], HT[:, c * 128:(c + 1) * 128],
                                     w[:, c * G3 + 2 * HS:(c + 1) * G3],
                                     start=False, stop=c == KC - 1)
            nc.tensor.matmul(NI[:], xl, wxr[:K, 2 * HS:G3],
                             start=True, stop=True)
            nc.tensor.matmul(Z[:], xl, wxr[:K, HS:2 * HS],
                             start=True, stop=HT is None)
            if HT is not None:
                for c in range(KC):
                    nc.tensor.matmul(Z[:], HT[:, c * 128:(c + 1) * 128],
                                     w[:, c * G3 + HS:c * G3 + 2 * HS],
                                     start=False, stop=c == KC - 1)
            return R, Z, NI, NH

        def gru_ew(R, Z, NI, NH, h_sb):
            """Half-tile pipelined elementwise GRU combine.
            h_sb: batch-major h (bf16, [128,512]) or None.
            S: sigma/tanh halves; V: rhn/npre/zn/hv halves; Pool: zc/zh.
            Returns hv [128,512] bf16."""
            r = sp.tile([128, HS], MMDT, tag="r")
            z = sp.tile([128, HS], MMDT, tag="z")
            rhn = sp.tile([128, HS], MMDT, tag="rhn")
            npre = sp.tile([128, HS], MMDT, tag="npre")
            n = sp.tile([128, HS], MMDT, tag="n")
            zc = sp.tile([128, HS], MMDT, tag="zc")
            zh = (sp.tile([128, HS], MMDT, tag="zh", name="zh")
                  if h_sb is not None else None)
            zn = sp.tile([128, HS], MMDT, tag="zn")
            hv = sp.tile([128, HS], MMDT, tag="hv")
            sl = [slice(0, HF), slice(HF, HS)]
            # S queue: sR.a sR.b sZ.a sZ.b tanh.a tanh.b
            for h in range(2):
                nc.scalar.activation(r[:, sl[h]], R[:, sl[h]], AF.Sigmoid)
            for h in range(2):
                nc.scalar.activation(z[:, sl[h]], Z[:, sl[h]], AF.Sigmoid)
            # Pool queue (off-chain z-path): zc = 1-z ; zh = z*h
            for h in range(2):
                nc.gpsimd.tensor_scalar(zc[:, sl[h]], z[:, sl[h]], -1.0, 1.0,
                                        mybir.AluOpType.mult,
                                        mybir.AluOpType.add)
                if h_sb is not None:
                    nc.gpsimd.tensor_mul(zh[:, sl[h]], z[:, sl[h]],
                                         h_sb[:, sl[h]])
            # V queue: rhn.a rhn.b npre.a npre.b ; then zn/hv after tanh
            for h in range(2):
                nc.vector.tensor_mul(rhn[:, sl[h]], r[:, sl[h]], NH[:, sl[h]])
            for h in range(2):
                nc.vector.tensor_add(npre[:, sl[h]], rhn[:, sl[h]],
                                     NI[:, sl[h]])
            for h in range(2):
                nc.scalar.activation(n[:, sl[h]], npre[:, sl[h]], AF.Tanh)
            for h in range(2):
                nc.vector.tensor_mul(zn[:, sl[h]], zc[:, sl[h]], n[:, sl[h]])
                if h_sb is not None:
                    nc.vector.tensor_add(hv[:, sl[h]], zn[:, sl[h]],
                                         zh[:, sl[h]])
            return hv if h_sb is not None else zn

        dumw = W["wgm"]

        def fill_pe(k):
            """k dependency-free 512-col matmuls into the transpose psum
            bank (free during elementwise windows). Keeps the PE p-state
            at full clock through gaps."""
            if k <= 0:
                return
            dum = ps_t.tile([128, HS], F32, tag="tp")
            for _ in range(k):
                nc.tensor.matmul(dum[:], W["eyeb"][:], dumw[:, 0:HS],
                                 start=True, stop=True)

        def diag_build(w, n):
            dg = dgp.tile([128, 128], MMDT, tag="diag")
            nc.gpsimd.tensor_scalar_mul(dg[:], eye[:],
                                        adjt[:, w * NV + n:w * NV + n + 1])
            return dg

        def diag_mm(Hp, dg, n, start, stop):
            nc.tensor.matmul(Hp[:], dg[:], gm_sb[n][:],
                             start=start, stop=stop)

        H_next = None

        for v in range(NV):
            if v == 0:
                hH = None
                HT = None
            else:
                # cast H_psum -> bf16 halves (V), then chunk-pipelined
                # transpose; copies on V
                hH = sp.tile([128, HS], MMDT, tag="hH")
                nc.vector.tensor_copy(hH[:, 0:HF], H_next[:, 0:HF])
                nc.vector.tensor_copy(hH[:, HF:HS], H_next[:, HF:HS])
                HT = transpose_block(halves(hH), "HT", 'v')

            R, Z, NI, NH = gru_mms(0, v, HT)
            # message terms for v+1 (sources < v available now)
            avail = list(range(v))
            half = (len(avail) + 1) // 2
            if v < NV - 1:
                H_next = ps_h.tile([128, HS], F32, tag="H")
                dgs = {i: diag_build(v + 1, i) for i in range(v + 1)}
                for i in avail[:half]:
                    diag_mm(H_next, dgs[i], i, start=(i == 0), stop=False)
            fill_pe(0 if v < 2 else max(2, 7 - len(avail[:half])))
            hv1 = gru_ew(R, Z, NI, NH, hH)
            hv1T = transpose_block(halves(hv1), "hv1T", 's')
            fill_pe(0 if v < 2 else 1)
            R, Z, NI, NH = gru_mms(1, v, hv1T)
            if v < NV - 1:
                for i in avail[half:]:
                    diag_mm(H_next, dgs[i], i, start=(i == 0), stop=False)
            fill_pe(0 if v < 2 else max(2, 7 - len(avail[half:])))
            hv = gru_ew(R, Z, NI, NH, hv1)
            hvT = transpose_block(halves(hv), "hvT", 's')
            fill_pe(0 if v < 2 else 1)

            if v < NV - 1:
                # gate/mapper for vertex v (feeds future messages)
                gatep = ps_m.tile([128, HS], F32, tag="psga")
                mapp = ps_m.tile([128, HS], F32, tag="psmp")
                vl = W["vsel"][:, v * P:(v + 1) * P]
                nc.tensor.matmul(gatep[:], vl, W["bgm"][:, 0:HS],
                                 start=True, stop=False)
                nc.tensor.matmul(mapp[:], vl, W["bgm"][:, HS:2 * HS],
                                 start=True, stop=False)
                for c in range(KC):
                    hl = hvT[:, c * 128:(c + 1) * 128]
                    last = c == KC - 1
                    nc.tensor.matmul(gatep[:], hl,
                                     W["wgm"][:, c * 2 * HS:c * 2 * HS + HS],
                                     start=False, stop=last)
                    nc.tensor.matmul(mapp[:], hl,
                                     W["wgm"][:, c * 2 * HS + HS:(c + 1) * 2 * HS],
                                     start=False, stop=last)
                gate = sp.tile([128, HS], MMDT, tag="gate")
                gmt = gmc.tile([128, HS], MMDT, tag=f"gm{v}")
                for h in range(2):
                    sl2 = slice(h * HF, (h + 1) * HF)
                    nc.scalar.activation(gate[:, sl2], gatep[:, sl2],
                                         AF.Sigmoid)
                    nc.vector.tensor_mul(gmt[:, sl2], gate[:, sl2],
                                         mapp[:, sl2])
                gm_sb.append(gmt)
                fill_pe(0 if v < 2 else 2)
                # last message term for step v+1 (needs gm_v)
                diag_mm(H_next, dgs[v], v, start=(v == 0), stop=True)
                fill_pe(0 if v < 2 else 2)
            else:
                # final FC: out = Hg @ Wfc + bfc   (mu | logvar)
                fcp = ps_m.tile([128, 2 * NZ], F32, tag="psga")
                nc.tensor.matmul(fcp[:], W["ones1"][:], W["bfc"][:, :],
                                 start=True, stop=False)
                for c in range(KC):
                    nc.tensor.matmul(fcp[:], hvT[:, c * 128:(c + 1) * 128],
                                     W["wfc"][:, c * 2 * NZ:(c + 1) * 2 * NZ],
                                     start=False, stop=(c == KC - 1))
                fc = sp.tile([128, 2 * NZ], F32, tag="fc")
                nc.scalar.copy(fc[:], fcp[:])
                nc.sync.dma_start(out_ap[:, :], fc[:])


def _host_prep(types, params, adj, gt_wi, gt_wh, gt_bi, gt_bh,
               gp_wi, gp_wh, gp_bi, gp_bh, gate_w, gate_b, mapper_w,
               fc1_w, fc1_b, fc2_w, fc2_b):
    """Pure layout prep: transposes/reshapes/one-hot + per-core sharding."""
    f = np.float32

    def chunked(a):  # [512, X] -> [128, 4*X] with K-chunks side by side
        X = a.shape[1]
        return np.ascontiguousarray(
            a.reshape(KC, 128, X).transpose(1, 0, 2).reshape(128, KC * X)).astype(f)

    b1 = np.concatenate([(gt_bi + gt_bh)[:2 * HS], gt_bi[2 * HS:]])
    b2 = np.concatenate([(gp_bi + gp_bh)[:2 * HS], gp_bi[2 * HS:]])
    shared = {
        "wht_t": chunked(gt_wh.T.astype(f)),
        "wht_p": chunked(gp_wh.T.astype(f)),
        "w1x": np.concatenate([gt_wi.T, b1[None, :]], 0).astype(f),
        "w2x": np.concatenate([gp_wi.T, b2[None, :]], 0).astype(f),
        "bhn2": np.concatenate([gt_bh[2 * HS:], gp_bh[2 * HS:]])[None, :].astype(f),
        "wgm": chunked(np.concatenate([gate_w[:, :HS].T, mapper_w[:, :HS].T], 1)),
        "bgm": np.stack([np.concatenate([gate_b + gate_w[:, HS + v],
                                         mapper_w[:, HS + v]])
                         for v in range(NV)]).astype(f),
        "vsel": np.repeat(np.eye(NV, dtype=f), P, axis=1),
        "wfc": chunked(np.concatenate([fc1_w.T, fc2_w.T], 1).astype(f)),
        "bfc": np.concatenate([fc1_b, fc2_b])[None, :].astype(f),
        "eye": np.eye(128, dtype=f),
        "eyeb": np.eye(128, dtype=f),
        "ones1": np.ones((1, 128), f),
    }
    oh = (types[:, :, None] == np.arange(NVT)[None, None, :]).astype(f)  # [B,NV,NVT]
    in_maps = []
    for c in range(NCORES):
        s = slice(c * P, (c + 1) * P)
        xt = oh[s].transpose(2, 1, 0).reshape(NVT, NV * P)           # [16, NV*P]
        xt1 = np.concatenate([xt, np.ones((1, NV * P), f)], 0)
        xp = params[s].transpose(2, 1, 0).reshape(FS, NV * P).astype(f)
        xp1 = np.concatenate([xp, np.ones((1, NV * P), f)], 0)
        m = dict(shared)
        m["xt1"] = np.ascontiguousarray(xt1)
        m["xp1"] = np.ascontiguousarray(xp1)
        m["adjt"] = np.ascontiguousarray(adj[s].reshape(P, NV * NV)).astype(f)
        in_maps.append(m)
    return in_maps


_NC_CACHE = {}


def _get_nc():
    key = str(MMDT)
    if key not in _NC_CACHE:
        _NC_CACHE[key] = build_bass()
    return _NC_CACHE[key]


F32_INPUTS = {"adjt", "eye"}


def kernel(**inputs):
    np_inputs = {k: np.asarray(v) for k, v in inputs.items()}
    in_maps = _host_prep(**np_inputs)
    npdt = mybir.dt.np(MMDT)
    if npdt != np.float32:
        in_maps = [{k: (v if k in F32_INPUTS else v.astype(npdt))
                    for k, v in m.items()} for m in in_maps]
    nc = _get_nc()
    res = run_bass_kernel_spmd(nc, in_maps, core_ids=list(range(NCORES)),
                               **_RUN_KWARGS)
    out = np.concatenate([res.results[c]["out"] for c in range(NCORES)], 0)
    _LAST_RESULT.clear()
    _LAST_RESULT.append(res)
    return out[:, :NZ], out[:, NZ:]


# test.py can set these to enable tracing / inspect results
_RUN_KWARGS = {}
_LAST_RESULT = []


# revision 7
# speedup vs baseline: 1.4613x; 1.4613x over previous
# Trainium2 Bass kernel for the DVAE encoder (nn_DVAE_24850680775463).
#
# Sharding: pure data-parallel. B=1024 graphs -> 8 cores x 128 graphs.
# Per core, the 128 graphs sit on the 128 SBUF partitions and the whole
# 16-vertex sequential scan runs on-chip.
#
# Structure (v2 — half-tile pipelined):
#  * gate/mapper products computed once per vertex (incremental cache);
#    vertex-id one-hot contribution folds into a per-vertex bias row.
#  * message H_v = sum_n adj[b,v,n] * gm[b,n,:] via PSUM-accumulated
#    matmuls with diag(adj[:,v,n]) stationary.
#  * GRU x-side + biases enter via small matmuls accumulating into the
#    same PSUM banks as the h-side matmuls.
#  * All elementwise / activation work is split into 256-wide halves so
#    the serial chain (sigma -> mul -> add -> tanh -> ...) pipelines across
#    the Scalar/Vector/GpSimd engines at half-tile granularity.
#  * z-path (1-z, z*h) runs early on GpSimd as soon as sigma(Z) lands, so
#    only zn/hv remain after tanh on the critical path.
#  * diag builds for messages run on GpSimd; transposes chunk-pipeline
#    with their PSUM->SBUF copies.
#  * small dependency-free PE fills keep the Tensor engine's p-state at
#    full clock through the elementwise windows.
# Matmuls default to bf16 (fp32 PSUM accumulation).

import os
import numpy as np

import concourse.bass as bass
import concourse.tile as tile
from concourse import bacc, mybir
from concourse.bass_utils import run_bass_kernel_spmd

AF = mybir.ActivationFunctionType
F32 = mybir.dt.float32

NCORES = 8
B, NV, NVT, FS, HS, NZ = 1024, 16, 16, 32, 512, 64
P = B // NCORES            # 128 graphs per core
G3 = 3 * HS                # 1536
K1 = NVT + 1               # 17  (one-hot + ones row)
K2 = FS + 1                # 33  (params + ones row)
KC = HS // 128             # 4 contraction chunks of the hidden dim
HF = HS // 2               # 256 half width

MMDT = {"f32r": mybir.dt.float32r, "f32": mybir.dt.float32,
        "bf16": mybir.dt.bfloat16}[os.environ.get("DVAE_MMDT", "bf16")]


def build_bass():
    nc = bacc.Bacc("TRN2", target_bir_lowering=False, debug=False)

    def inp(name, shape, dt=None):
        return nc.dram_tensor(name, shape, dt or MMDT,
                              kind="ExternalInput").ap()

    d = {
        "wht_t": inp("wht_t", [128, KC * G3]),
        "wht_p": inp("wht_p", [128, KC * G3]),
        "w1x":   inp("w1x",   [K1, G3]),
        "w2x":   inp("w2x",   [K2, G3]),
        "xt1":   inp("xt1",   [K1, NV * P]),
        "xp1":   inp("xp1",   [K2, NV * P]),
        "bhn2":  inp("bhn2",  [1, 2 * HS]),
        "wgm":   inp("wgm",   [128, KC * 2 * HS]),
        "bgm":   inp("bgm",   [NV, 2 * HS]),
        "vsel":  inp("vsel",  [NV, NV * P]),
        "adjt":  inp("adjt",  [P, NV * NV], F32),
        "wfc":   inp("wfc",   [128, KC * 2 * NZ]),
        "bfc":   inp("bfc",   [1, 2 * NZ]),
        "eye":   inp("eye",   [128, 128], F32),
        "eyeb":  inp("eyeb",  [128, 128]),
        "ones1": inp("ones1", [1, 128]),
    }
    out_ap = nc.dram_tensor("out", [P, 2 * NZ], mybir.dt.float32, kind="ExternalOutput").ap()

    with tile.TileContext(nc) as tc:
        _body(tc, d, out_ap)
    nc.compile()
    return nc


def _body(tc, d, out_ap):
    nc = tc.nc
    from contextlib import ExitStack
    with ExitStack() as ctx:
        wp = ctx.enter_context(tc.tile_pool(name="w", bufs=1))
        sp = ctx.enter_context(tc.tile_pool(name="s", bufs=1))
        dgp = ctx.enter_context(tc.tile_pool(name="dg", bufs=16))
        gmc = ctx.enter_context(tc.tile_pool(name="gmc", bufs=1))
        ps_h = ctx.enter_context(tc.tile_pool(name="psh", bufs=1, space="PSUM"))
        ps_g = ctx.enter_context(tc.tile_pool(name="psg", bufs=4, space="PSUM"))
        ps_t = ctx.enter_context(tc.tile_pool(name="pst", bufs=1, space="PSUM"))
        ps_m = ctx.enter_context(tc.tile_pool(name="psm", bufs=1, space="PSUM"))

        # ---- persistent weights / constants ----
        order = ["ones1", "w1x", "xt1", "bhn2", "eye", "eyeb", "adjt", "xp1",
                 "w2x", "wgm", "wht_p", "vsel", "bgm", "wht_t", "wfc", "bfc"]
        W = {}
        for name, ap in sorted(d.items(), key=lambda kv: order.index(kv[0])):
            t = wp.tile(list(ap.shape), ap.dtype, tag=name)
            nc.sync.dma_start(t[:], ap[:, :])
            W[name] = t

        wht = {0: W["wht_t"], 1: W["wht_p"]}
        wx = {0: W["w1x"], 1: W["w2x"]}
        xs = {0: W["xt1"], 1: W["xp1"]}
        kx = {0: K1, 1: K2}
        eye = W["eye"]
        adjt = W["adjt"]

        gm_sb = []          # cached gate*mapped per vertex, [P, HS] each

        def halves(t):
            return (t[:, 0:HF], t[:, HF:HS])

        def transpose_block(src_ab, tag, copy_eng):
            """src_ab: two [128,256] bf16 batch-major halves (or one [128,512]
            tile's halves). PE-transpose chunk-by-chunk; copies pipelined.
            copy_eng: 'v' or 's' for the PSUM->SBUF chunk copies."""
            tp = ps_t.tile([128, HS], MMDT, tag="tp")
            dst = sp.tile([128, HS], MMDT, tag=tag)
            for c in range(KC):
                src = src_ab[c // 2]
                nc.tensor.transpose(tp[:, c * 128:(c + 1) * 128],
                                    src[:, (c % 2) * 128:(c % 2 + 1) * 128],
                                    W["eyeb"][:])
            for h in range(2):
                sl = slice(h * HF, (h + 1) * HF)
                if copy_eng == 'v':
                    nc.vector.tensor_copy(dst[:, sl], tp[:, sl])
                else:
                    nc.scalar.copy(dst[:, sl], tp[:, sl])
            return dst

        def gru_mms(g, v, HT):
            """Emit all matmuls of GRU g (0=type,1=param) for vertex v.
            HT: feature-major hidden state [128, 512] or None (h=0).
            Gate emission order: R first (its sigma leads the chain), then
            NH (rhn needs it 2nd), NI, Z."""
            K = kx[g]
            xl = xs[g][:K, v * P:(v + 1) * P]      # lhsT [K,128]
            wxr = wx[g]
            w = wht[g]
            R = ps_g.tile([128, HS], F32, tag="g")
            NH = ps_g.tile([128, HS], F32, tag="g")
            NI = ps_g.tile([128, HS], F32, tag="g")
            Z = ps_g.tile([128, HS], F32, tag="g")
            nc.tensor.matmul(R[:], xl, wxr[:K, 0:HS],
                             start=True, stop=HT is None)
            if HT is not None:
                for c in range(KC):
                    nc.tensor.matmul(R[:], HT[:, c * 128:(c + 1) * 128],
                                     w[:, c * G3:c * G3 + HS],
                                     start=False, stop=c == KC - 1)
            nc.tensor.matmul(NH[:], W["ones1"][:],
                             W["bhn2"][0:1, g * HS:(g + 1) * HS],
                             start=True, stop=HT is None)
            if HT is not None:
                for c in range(KC):
                    nc.tensor.matmul(NH[:], HT[:, c * 128:(c + 1) * 128],
                                     w[:, c * G3 + 2 * HS:(c + 1) * G3],
                                     start=False, stop=c == KC - 1)
            nc.tensor.matmul(NI[:], xl, wxr[:K, 2 * HS:G3],
                             start=True, stop=True)
            nc.tensor.matmul(Z[:], xl, wxr[:K, HS:2 * HS],
                             start=True, stop=HT is None)
            if HT is not None:
                for c in range(KC):
                    nc.tensor.matmul(Z[:], HT[:, c * 128:(c + 1) * 128],
                                     w[:, c * G3 + HS:c * G3 + 2 * HS],
                                     start=False, stop=c == KC - 1)
            return R, Z, NI, NH

        def gru_ew(R, Z, NI, NH, h_sb):
            """Half-tile pipelined elementwise GRU combine.
            h_sb: batch-major h (bf16, [128,512]) or None.
            S: sigma/tanh halves; V: rhn/npre/zn/hv halves; Pool: zc/zh.
            Returns hv [128,512] bf16."""
            r = sp.tile([128, HS], MMDT, tag="r")
            z = sp.tile([128, HS], MMDT, tag="z")
            rhn = sp.tile([128, HS], MMDT, tag="rhn")
            npre = sp.tile([128, HS], MMDT, tag="npre")
            n = sp.tile([128, HS], MMDT, tag="n")
            zc = sp.tile([128, HS], MMDT, tag="zc")
            zh = (sp.tile([128, HS], MMDT, tag="zh", name="zh")
                  if h_sb is not None else None)
            zn = sp.tile([128, HS], MMDT, tag="zn")
            hv = sp.tile([128, HS], MMDT, tag="hv")
            sl = [slice(0, HF), slice(HF, HS)]
            # S queue: sR.a sR.b sZ.a sZ.b tanh.a tanh.b
            for h in range(2):
                nc.scalar.activation(r[:, sl[h]], R[:, sl[h]], AF.Sigmoid)
            for h in range(2):
                nc.scalar.activation(z[:, sl[h]], Z[:, sl[h]], AF.Sigmoid)
            # off-chain z-path, full width on V: zc = 1-z ; zh = z*h
            nc.vector.tensor_scalar(zc[:], z[:], -1.0, 1.0,
                                    mybir.AluOpType.mult,
                                    mybir.AluOpType.add)
            if h_sb is not None:
                nc.vector.tensor_mul(zh[:], z[:], h_sb[:])
            # V queue: rhn.a rhn.b npre.a npre.b ; then zn/hv after tanh
            for h in range(2):
                nc.vector.tensor_mul(rhn[:, sl[h]], r[:, sl[h]], NH[:, sl[h]])
            for h in range(2):
                nc.vector.tensor_add(npre[:, sl[h]], rhn[:, sl[h]],
                                     NI[:, sl[h]])
            for h in range(2):
                nc.scalar.activation(n[:, sl[h]], npre[:, sl[h]], AF.Tanh)
            for h in range(2):
                nc.vector.tensor_mul(zn[:, sl[h]], zc[:, sl[h]], n[:, sl[h]])
                if h_sb is not None:
                    nc.vector.tensor_add(hv[:, sl[h]], zn[:, sl[h]],
                                         zh[:, sl[h]])
            return hv if h_sb is not None else zn

        dumw = W["wgm"]

        def fill_pe(k):
            """k dependency-free 512-col matmuls into the transpose psum
            bank (free during elementwise windows). Keeps the PE p-state
            at full clock through gaps."""
            if k <= 0:
                return
            dum = ps_t.tile([128, HS], F32, tag="tp")
            for _ in range(k):
                nc.tensor.matmul(dum[:], W["eyeb"][:], dumw[:, 0:HS],
                                 start=True, stop=True)

        def diag_build(w, n):
            # scalar-engine diag build: dg = eye * adj[:, w, n] (per-partition
            # scale) — keeps the Vector engine free for the GRU chain
            dg = dgp.tile([128, 128], MMDT, tag="diag")
            nc.scalar.activation(dg[:], eye[:], AF.Copy,
                                 scale=adjt[:, w * NV + n:w * NV + n + 1])
            return dg

        def diag_mm(Hp, dg, n, start, stop):
            nc.tensor.matmul(Hp[:], dg[:], gm_sb[n][:],
                             start=start, stop=stop)

        H_next = None

        for v in range(NV):
            if v == 0:
                hH = None
                HT = None
            else:
                # cast H_psum -> bf16 halves (S), then chunk-pipelined
                # transpose; copies on V
                hH = sp.tile([128, HS], MMDT, tag="hH")
                nc.scalar.copy(hH[:, 0:HF], H_next[:, 0:HF])
                nc.scalar.copy(hH[:, HF:HS], H_next[:, HF:HS])
                HT = transpose_block(halves(hH), "HT", 'v')

            R, Z, NI, NH = gru_mms(0, v, HT)
            # message terms for v+1 (sources < v available now)
            avail = list(range(v))
            half = (len(avail) + 1) // 2
            if v < NV - 1:
                H_next = ps_h.tile([128, HS], F32, tag="H")
                dgs = {i: diag_build(v + 1, i) for i in range(v + 1)}
                for i in avail[:half]:
                    diag_mm(H_next, dgs[i], i, start=(i == 0), stop=False)
            fill_pe(0 if v < 2 else max(2, 7 - len(avail[:half])))
            hv1 = gru_ew(R, Z, NI, NH, hH)
            hv1T = transpose_block(halves(hv1), "hv1T", 's')
            fill_pe(0 if v < 2 else 1)
            R, Z, NI, NH = gru_mms(1, v, hv1T)
            if v < NV - 1:
                for i in avail[half:]:
                    diag_mm(H_next, dgs[i], i, start=(i == 0), stop=False)
            fill_pe(0 if v < 2 else max(2, 7 - len(avail[half:])))
            hv = gru_ew(R, Z, NI, NH, hv1)
            hvT = transpose_block(halves(hv), "hvT", 's')
            fill_pe(0 if v < 2 else 1)

            if v < NV - 1:
                # gate/mapper for vertex v (feeds future messages)
                gatep = ps_m.tile([128, HS], F32, tag="psga")
                mapp = ps_m.tile([128, HS], F32, tag="psmp")
                vl = W["vsel"][:, v * P:(v + 1) * P]
                nc.tensor.matmul(gatep[:], vl, W["bgm"][:, 0:HS],
                                 start=True, stop=False)
                nc.tensor.matmul(mapp[:], vl, W["bgm"][:, HS:2 * HS],
                                 start=True, stop=False)
                for c in range(KC):
                    hl = hvT[:, c * 128:(c + 1) * 128]
                    last = c == KC - 1
                    nc.tensor.matmul(gatep[:], hl,
                                     W["wgm"][:, c * 2 * HS:c * 2 * HS + HS],
                                     start=False, stop=last)
                    nc.tensor.matmul(mapp[:], hl,
                                     W["wgm"][:, c * 2 * HS + HS:(c + 1) * 2 * HS],
                                     start=False, stop=last)
                gate = sp.tile([128, HS], MMDT, tag="gate")
                gmt = gmc.tile([128, HS], MMDT, tag=f"gm{v}")
                for h in range(2):
                    sl2 = slice(h * HF, (h + 1) * HF)
                    nc.scalar.activation(gate[:, sl2], gatep[:, sl2],
                                         AF.Sigmoid)
                    nc.vector.tensor_mul(gmt[:, sl2], gate[:, sl2],
                                         mapp[:, sl2])
                gm_sb.append(gmt)
                fill_pe(0 if v < 2 else 2)
                # last message term for step v+1 (needs gm_v)
                diag_mm(H_next, dgs[v], v, start=(v == 0), stop=True)
                fill_pe(0 if v < 2 else 2)
            else:
                # final FC: out = Hg @ Wfc + bfc   (mu | logvar)
                fcp = ps_m.tile([128, 2 * NZ], F32, tag="psga")
                nc.tensor.matmul(fcp[:], W["ones1"][:], W["bfc"][:, :],
                                 start=True, stop=False)
                for c in range(KC):
                    nc.tensor.matmul(fcp[:], hvT[:, c * 128:(c + 1) * 128],
                                     W["wfc"][:, c * 2 * NZ:(c + 1) * 2 * NZ],
                                     start=False, stop=(c == KC - 1))
                fc = sp.tile([128, 2 * NZ], F32, tag="fc")
                nc.scalar.copy(fc[:], fcp[:])
                nc.sync.dma_start(out_ap[:, :], fc[:])


def _host_prep(types, params, adj, gt_wi, gt_wh, gt_bi, gt_bh,
               gp_wi, gp_wh, gp_bi, gp_bh, gate_w, gate_b, mapper_w,
               fc1_w, fc1_b, fc2_w, fc2_b):
    """Pure layout prep: transposes/reshapes/one-hot + per-core sharding."""
    f = np.float32

    def chunked(a):  # [512, X] -> [128, 4*X] with K-chunks side by side
        X = a.shape[1]
        return np.ascontiguousarray(
            a.reshape(KC, 128, X).transpose(1, 0, 2).reshape(128, KC * X)).astype(f)

    b1 = np.concatenate([(gt_bi + gt_bh)[:2 * HS], gt_bi[2 * HS:]])
    b2 = np.concatenate([(gp_bi + gp_bh)[:2 * HS], gp_bi[2 * HS:]])
    shared = {
        "wht_t": chunked(gt_wh.T.astype(f)),
        "wht_p": chunked(gp_wh.T.astype(f)),
        "w1x": np.concatenate([gt_wi.T, b1[None, :]], 0).astype(f),
        "w2x": np.concatenate([gp_wi.T, b2[None, :]], 0).astype(f),
        "bhn2": np.concatenate([gt_bh[2 * HS:], gp_bh[2 * HS:]])[None, :].astype(f),
        "wgm": chunked(np.concatenate([gate_w[:, :HS].T, mapper_w[:, :HS].T], 1)),
        "bgm": np.stack([np.concatenate([gate_b + gate_w[:, HS + v],
                                         mapper_w[:, HS + v]])
                         for v in range(NV)]).astype(f),
        "vsel": np.repeat(np.eye(NV, dtype=f), P, axis=1),
        "wfc": chunked(np.concatenate([fc1_w.T, fc2_w.T], 1).astype(f)),
        "bfc": np.concatenate([fc1_b, fc2_b])[None, :].astype(f),
        "eye": np.eye(128, dtype=f),
        "eyeb": np.eye(128, dtype=f),
        "ones1": np.ones((1, 128), f),
    }
    oh = (types[:, :, None] == np.arange(NVT)[None, None, :]).astype(f)  # [B,NV,NVT]
    in_maps = []
    for c in range(NCORES):
        s = slice(c * P, (c + 1) * P)
        xt = oh[s].transpose(2, 1, 0).reshape(NVT, NV * P)           # [16, NV*P]
        xt1 = np.concatenate([xt, np.ones((1, NV * P), f)], 0)
        xp = params[s].transpose(2, 1, 0).reshape(FS, NV * P).astype(f)
        xp1 = np.concatenate([xp, np.ones((1, NV * P), f)], 0)
        m = dict(shared)
        m["xt1"] = np.ascontiguousarray(xt1)
        m["xp1"] = np.ascontiguousarray(xp1)
        m["adjt"] = np.ascontiguousarray(adj[s].reshape(P, NV * NV)).astype(f)
        in_maps.append(m)
    return in_maps


_NC_CACHE = {}


def _get_nc():
    key = str(MMDT)
    if key not in _NC_CACHE:
        _NC_CACHE[key] = build_bass()
    return _NC_CACHE[key]


F32_INPUTS = {"adjt", "eye"}


def kernel(**inputs):
    np_inputs = {k: np.asarray(v) for k, v in inputs.items()}
    in_maps = _host_prep(**np_inputs)
    npdt = mybir.dt.np(MMDT)
    if npdt != np.float32:
        in_maps = [{k: (v if k in F32_INPUTS else v.astype(npdt))
                    for k, v in m.items()} for m in in_maps]
    nc = _get_nc()
    res = run_bass_kernel_spmd(nc, in_maps, core_ids=list(range(NCORES)),
                               **_RUN_KWARGS)
    out = np.concatenate([res.results[c]["out"] for c in range(NCORES)], 0)
    _LAST_RESULT.clear()
    _LAST_RESULT.append(res)
    return out[:, :NZ], out[:, NZ:]


# test.py can set these to enable tracing / inspect results
_RUN_KWARGS = {}
_LAST_RESULT = []


# revision 14
# speedup vs baseline: 1.5168x; 1.0380x over previous
# Trainium2 Bass kernel for the DVAE encoder (nn_DVAE_24850680775463).
#
# Sharding: pure data-parallel. B=1024 graphs -> 8 cores x 128 graphs.
# Per core, the 128 graphs sit on the 128 SBUF partitions and the whole
# 16-vertex sequential scan runs on-chip.
#
# Structure (v2 — half-tile pipelined):
#  * gate/mapper products computed once per vertex (incremental cache);
#    vertex-id one-hot contribution folds into a per-vertex bias row.
#  * message H_v = sum_n adj[b,v,n] * gm[b,n,:] via PSUM-accumulated
#    matmuls with diag(adj[:,v,n]) stationary.
#  * GRU x-side + biases enter via small matmuls accumulating into the
#    same PSUM banks as the h-side matmuls.
#  * All elementwise / activation work is split into 256-wide halves so
#    the serial chain (sigma -> mul -> add -> tanh -> ...) pipelines across
#    the Scalar/Vector/GpSimd engines at half-tile granularity.
#  * z-path (1-z, z*h) runs early on GpSimd as soon as sigma(Z) lands, so
#    only zn/hv remain after tanh on the critical path.
#  * diag builds for messages run on GpSimd; transposes chunk-pipeline
#    with their PSUM->SBUF copies.
#  * small dependency-free PE fills keep the Tensor engine's p-state at
#    full clock through the elementwise windows.
# Matmuls default to bf16 (fp32 PSUM accumulation).

import os
import numpy as np

import concourse.bass as bass
import concourse.tile as tile
from concourse import bacc, mybir
from concourse.bass_utils import run_bass_kernel_spmd

AF = mybir.ActivationFunctionType
F32 = mybir.dt.float32

NCORES = 8
B, NV, NVT, FS, HS, NZ = 1024, 16, 16, 32, 512, 64
P = B // NCORES            # 128 graphs per core
G3 = 3 * HS                # 1536
K1 = NVT + 1               # 17  (one-hot + ones row)
K2 = FS + 1                # 33  (params + ones row)
KC = HS // 128             # 4 contraction chunks of the hidden dim
HF = HS // 2               # 256 half width

MMDT = {"f32r": mybir.dt.float32r, "f32": mybir.dt.float32,
        "bf16": mybir.dt.bfloat16}[os.environ.get("DVAE_MMDT", "bf16")]


def build_bass():
    nc = bacc.Bacc("TRN2", target_bir_lowering=False, debug=False)

    def inp(name, shape, dt=None):
        return nc.dram_tensor(name, shape, dt or MMDT,
                              kind="ExternalInput").ap()

    d = {
        "wht_t": inp("wht_t", [128, KC * G3]),
        "wht_p": inp("wht_p", [128, KC * G3]),
        "w1x":   inp("w1x",   [K1, G3]),
        "w2x":   inp("w2x",   [K2, G3]),
        "xt1":   inp("xt1",   [K1, NV * P]),
        "xp1":   inp("xp1",   [K2, NV * P]),
        "bhn2":  inp("bhn2",  [1, 2 * HS]),
        "wgm":   inp("wgm",   [128, KC * 2 * HS]),
        "bgm":   inp("bgm",   [NV, 2 * HS]),
        "vsel":  inp("vsel",  [NV, NV * P]),
        "adjt":  inp("adjt",  [P, NV * NV], F32),
        "wfc":   inp("wfc",   [128, KC * 2 * NZ]),
        "bfc":   inp("bfc",   [1, 2 * NZ]),
        "eye":   inp("eye",   [128, 128], F32),
        "eyeb":  inp("eyeb",  [128, 128]),
        "ones1": inp("ones1", [1, 128]),
    }
    out_ap = nc.dram_tensor("out", [P, 2 * NZ], mybir.dt.float32, kind="ExternalOutput").ap()

    with tile.TileContext(nc) as tc:
        _body(tc, d, out_ap)
    nc.compile()
    return nc


def _body(tc, d, out_ap):
    nc = tc.nc
    from contextlib import ExitStack
    with ExitStack() as ctx:
        wp = ctx.enter_context(tc.tile_pool(name="w", bufs=1))
        sp = ctx.enter_context(tc.tile_pool(name="s", bufs=1))
        dgp = ctx.enter_context(tc.tile_pool(name="dg", bufs=16))
        gmc = ctx.enter_context(tc.tile_pool(name="gmc", bufs=1))
        ps_h = ctx.enter_context(tc.tile_pool(name="psh", bufs=1, space="PSUM"))
        ps_g = ctx.enter_context(tc.tile_pool(name="psg", bufs=4, space="PSUM"))
        ps_t = ctx.enter_context(tc.tile_pool(name="pst", bufs=1, space="PSUM"))
        ps_m = ctx.enter_context(tc.tile_pool(name="psm", bufs=1, space="PSUM"))

        # ---- persistent weights / constants ----
        order = ["ones1", "w1x", "xt1", "bhn2", "eye", "eyeb", "adjt", "xp1",
                 "w2x", "wgm", "wht_p", "vsel", "bgm", "wht_t", "wfc", "bfc"]
        W = {}
        for name, ap in sorted(d.items(), key=lambda kv: order.index(kv[0])):
            t = wp.tile(list(ap.shape), ap.dtype, tag=name)
            nc.sync.dma_start(t[:], ap[:, :])
            W[name] = t

        wht = {0: W["wht_t"], 1: W["wht_p"]}
        wx = {0: W["w1x"], 1: W["w2x"]}
        xs = {0: W["xt1"], 1: W["xp1"]}
        kx = {0: K1, 1: K2}
        eye = W["eye"]
        adjt = W["adjt"]

        gm_sb = []          # cached gate*mapped per vertex, [P, HS] each

        def halves(t):
            return (t[:, 0:HF], t[:, HF:HS])

        def transpose_block(src_ab, tag, copy_eng):
            """src_ab: two [128,256] bf16 batch-major halves (or one [128,512]
            tile's halves). PE-transpose chunk-by-chunk; copies pipelined.
            copy_eng: 'v' or 's' for the PSUM->SBUF chunk copies."""
            tp = ps_t.tile([128, HS], MMDT, tag="tp")
            dst = sp.tile([128, HS], MMDT, tag=tag)
            for c in range(KC):
                src = src_ab[c // 2]
                nc.tensor.transpose(tp[:, c * 128:(c + 1) * 128],
                                    src[:, (c % 2) * 128:(c % 2 + 1) * 128],
                                    W["eyeb"][:])
            for h in range(2):
                sl = slice(h * HF, (h + 1) * HF)
                if copy_eng == 'v':
                    nc.vector.tensor_copy(dst[:, sl], tp[:, sl])
                else:
                    nc.scalar.copy(dst[:, sl], tp[:, sl])
            return dst

        def gru_mms(g, v, HT):
            """Emit all matmuls of GRU g (0=type,1=param) for vertex v.
            HT: feature-major hidden state [128, 512] or None (h=0).
            Gate emission order: R first (its sigma leads the chain), then
            NH (rhn needs it 2nd), NI, Z."""
            K = kx[g]
            xl = xs[g][:K, v * P:(v + 1) * P]      # lhsT [K,128]
            wxr = wx[g]
            w = wht[g]
            R = ps_g.tile([128, HS], F32, tag="g")
            NH = ps_g.tile([128, HS], F32, tag="g")
            NI = ps_g.tile([128, HS], F32, tag="g")
            Z = ps_g.tile([128, HS], F32, tag="g")
            # all x-side / bias matmuls first: they have no dependency on HT,
            # so they execute during the preceding elementwise window while
            # the h-side matmuls below wait on the transpose
            nc.tensor.matmul(R[:], xl, wxr[:K, 0:HS],
                             start=True, stop=HT is None)
            nc.tensor.matmul(NH[:], W["ones1"][:],
                             W["bhn2"][0:1, g * HS:(g + 1) * HS],
                             start=True, stop=HT is None)
            nc.tensor.matmul(NI[:], xl, wxr[:K, 2 * HS:G3],
                             start=True, stop=True)
            nc.tensor.matmul(Z[:], xl, wxr[:K, HS:2 * HS],
                             start=True, stop=HT is None)
            if HT is not None:
                for c in range(KC):
                    nc.tensor.matmul(R[:], HT[:, c * 128:(c + 1) * 128],
                                     w[:, c * G3:c * G3 + HS],
                                     start=False, stop=c == KC - 1)
                for c in range(KC):
                    nc.tensor.matmul(NH[:], HT[:, c * 128:(c + 1) * 128],
                                     w[:, c * G3 + 2 * HS:(c + 1) * G3],
                                     start=False, stop=c == KC - 1)
                for c in range(KC):
                    nc.tensor.matmul(Z[:], HT[:, c * 128:(c + 1) * 128],
                                     w[:, c * G3 + HS:c * G3 + 2 * HS],
                                     start=False, stop=c == KC - 1)
            return R, Z, NI, NH

        def gru_ew(R, Z, NI, NH, h_sb):
            """Half-tile pipelined elementwise GRU combine.
            h_sb: batch-major h (bf16, [128,512]) or None.
            S: sigma/tanh halves; V: rhn/npre/zn/hv halves; Pool: zc/zh.
            Returns hv [128,512] bf16."""
            r = sp.tile([128, HS], MMDT, tag="r")
            z = sp.tile([128, HS], MMDT, tag="z")
            rhn = sp.tile([128, HS], MMDT, tag="rhn")
            npre = sp.tile([128, HS], MMDT, tag="npre")
            n = sp.tile([128, HS], MMDT, tag="n")
            zc = sp.tile([128, HS], MMDT, tag="zc")
            zh = (sp.tile([128, HS], MMDT, tag="zh", name="zh")
                  if h_sb is not None else None)
            zn = sp.tile([128, HS], MMDT, tag="zn")
            hv = sp.tile([128, HS], MMDT, tag="hv")
            sl = [slice(0, HF), slice(HF, HS)]
            # S queue: sR.a sR.b sZ tanh.a tanh.b (sZ full width: z is
            # consumed off-chain, halves only pay the ACT fixed cost)
            for h in range(2):
                nc.scalar.activation(r[:, sl[h]], R[:, sl[h]], AF.Sigmoid)
            nc.scalar.activation(z[:], Z[:], AF.Sigmoid)
            # off-chain z-path, full width on V: zc = 1-z ; zh = z*h
            nc.vector.tensor_scalar(zc[:], z[:], -1.0, 1.0,
                                    mybir.AluOpType.mult,
                                    mybir.AluOpType.add)
            if h_sb is not None:
                nc.vector.tensor_mul(zh[:], z[:], h_sb[:])
            # V queue: rhn.a rhn.b npre.a npre.b ; then zn/hv after tanh
            for h in range(2):
                nc.vector.tensor_mul(rhn[:, sl[h]], r[:, sl[h]], NH[:, sl[h]])
            for h in range(2):
                nc.vector.tensor_add(npre[:, sl[h]], rhn[:, sl[h]],
                                     NI[:, sl[h]])
            for h in range(2):
                nc.scalar.activation(n[:, sl[h]], npre[:, sl[h]], AF.Tanh)
            for h in range(2):
                nc.vector.tensor_mul(zn[:, sl[h]], zc[:, sl[h]], n[:, sl[h]])
                if h_sb is not None:
                    nc.vector.tensor_add(hv[:, sl[h]], zn[:, sl[h]],
                                         zh[:, sl[h]])
            return hv if h_sb is not None else zn

        dumw = W["wgm"]

        def fill_pe(k):
            """k dependency-free 512-col matmuls into the transpose psum
            bank (free during elementwise windows). Keeps the PE p-state
            at full clock through gaps."""
            if k <= 0:
                return
            dum = ps_t.tile([128, HS], F32, tag="tp")
            for _ in range(k):
                nc.tensor.matmul(dum[:], W["eyeb"][:], dumw[:, 0:HS],
                                 start=True, stop=True)

        def diag_build(w, n):
            dg = dgp.tile([128, 128], MMDT, tag="diag")
            nc.vector.tensor_scalar_mul(dg[:], eye[:],
                                        adjt[:, w * NV + n:w * NV + n + 1])
            return dg

        def diag_mm(Hp, dg, n, start, stop):
            nc.tensor.matmul(Hp[:], dg[:], gm_sb[n][:],
                             start=start, stop=stop)

        H_next = None

        for v in range(NV):
            if v == 0:
                hH = None
                HT = None
            else:
                # cast H_psum -> bf16 (S), then chunk-pipelined transpose;
                # copies on V
                hH = sp.tile([128, HS], MMDT, tag="hH")
                nc.scalar.copy(hH[:], H_next[:])
                HT = transpose_block(halves(hH), "HT", 'v')

            R, Z, NI, NH = gru_mms(0, v, HT)
            # message terms for v+1 (sources < v available now)
            avail = list(range(v))
            half = (len(avail) + 1) // 2
            if v < NV - 1:
                H_next = ps_h.tile([128, HS], F32, tag="H")
                dgs = {i: diag_build(v + 1, i) for i in range(v + 1)}
                for i in avail[:half]:
                    diag_mm(H_next, dgs[i], i, start=(i == 0), stop=False)
            fill_pe(0 if v < 2 else max(2, 7 - len(avail[:half])))
            hv1 = gru_ew(R, Z, NI, NH, hH)
            hv1T = transpose_block(halves(hv1), "hv1T", 'v')
            fill_pe(0 if v < 2 else 3)
            R, Z, NI, NH = gru_mms(1, v, hv1T)
            if v < NV - 1:
                for i in avail[half:]:
                    diag_mm(H_next, dgs[i], i, start=(i == 0), stop=False)
            fill_pe(0 if v < 2 else max(2, 7 - len(avail[half:])))
            hv = gru_ew(R, Z, NI, NH, hv1)
            hvT = transpose_block(halves(hv), "hvT", 's')
            fill_pe(0 if v < 2 else 2)

            if v < NV - 1:
                # gate/mapper for vertex v (feeds future messages)
                gatep = ps_m.tile([128, HS], F32, tag="psga")
                mapp = ps_m.tile([128, HS], F32, tag="psmp")
                vl = W["vsel"][:, v * P:(v + 1) * P]
                nc.tensor.matmul(gatep[:], vl, W["bgm"][:, 0:HS],
                                 start=True, stop=False)
                nc.tensor.matmul(mapp[:], vl, W["bgm"][:, HS:2 * HS],
                                 start=True, stop=False)
                for c in range(KC):
                    hl = hvT[:, c * 128:(c + 1) * 128]
                    last = c == KC - 1
                    nc.tensor.matmul(gatep[:], hl,
                                     W["wgm"][:, c * 2 * HS:c * 2 * HS + HS],
                                     start=False, stop=last)
                    nc.tensor.matmul(mapp[:], hl,
                                     W["wgm"][:, c * 2 * HS + HS:(c + 1) * 2 * HS],
                                     start=False, stop=last)
                gate = sp.tile([128, HS], MMDT, tag="gate")
                gmt = gmc.tile([128, HS], MMDT, tag=f"gm{v}")
                nc.scalar.activation(gate[:], gatep[:], AF.Sigmoid)
                nc.vector.tensor_mul(gmt[:], gate[:], mapp[:])
                gm_sb.append(gmt)
                fill_pe(0 if v < 2 else 3)
                # last message term for step v+1 (needs gm_v)
                diag_mm(H_next, dgs[v], v, start=(v == 0), stop=True)
                fill_pe(0 if v < 2 else 4)
            else:
                # final FC: out = Hg @ Wfc + bfc   (mu | logvar)
                fcp = ps_m.tile([128, 2 * NZ], F32, tag="psga")
                nc.tensor.matmul(fcp[:], W["ones1"][:], W["bfc"][:, :],
                                 start=True, stop=False)
                for c in range(KC):
                    nc.tensor.matmul(fcp[:], hvT[:, c * 128:(c + 1) * 128],
                                     W["wfc"][:, c * 2 * NZ:(c + 1) * 2 * NZ],
                                     start=False, stop=(c == KC - 1))
                fc = sp.tile([128, 2 * NZ], F32, tag="fc")
                nc.scalar.copy(fc[:], fcp[:])
                nc.sync.dma_start(out_ap[:, :], fc[:])


def _host_prep(types, params, adj, gt_wi, gt_wh, gt_bi, gt_bh,
               gp_wi, gp_wh, gp_bi, gp_bh, gate_w, gate_b, mapper_w,
               fc1_w, fc1_b, fc2_w, fc2_b):
    """Pure layout prep: transposes/reshapes/one-hot + per-core sharding."""
    f = np.float32

    def chunked(a):  # [512, X] -> [128, 4*X] with K-chunks side by side
        X = a.shape[1]
        return np.ascontiguousarray(
            a.reshape(KC, 128, X).transpose(1, 0, 2).reshape(128, KC * X)).astype(f)

    b1 = np.concatenate([(gt_bi + gt_bh)[:2 * HS], gt_bi[2 * HS:]])
    b2 = np.concatenate([(gp_bi + gp_bh)[:2 * HS], gp_bi[2 * HS:]])
    shared = {
        "wht_t": chunked(gt_wh.T.astype(f)),
        "wht_p": chunked(gp_wh.T.astype(f)),
        "w1x": np.concatenate([gt_wi.T, b1[None, :]], 0).astype(f),
        "w2x": np.concatenate([gp_wi.T, b2[None, :]], 0).astype(f),
        "bhn2": np.concatenate([gt_bh[2 * HS:], gp_bh[2 * HS:]])[None, :].astype(f),
        "wgm": chunked(np.concatenate([gate_w[:, :HS].T, mapper_w[:, :HS].T], 1)),
        "bgm": np.stack([np.concatenate([gate_b + gate_w[:, HS + v],
                                         mapper_w[:, HS + v]])
                         for v in range(NV)]).astype(f),
        "vsel": np.repeat(np.eye(NV, dtype=f), P, axis=1),
        "wfc": chunked(np.concatenate([fc1_w.T, fc2_w.T], 1).astype(f)),
        "bfc": np.concatenate([fc1_b, fc2_b])[None, :].astype(f),
        "eye": np.eye(128, dtype=f),
        "eyeb": np.eye(128, dtype=f),
        "ones1": np.ones((1, 128), f),
    }
    oh = (types[:, :, None] == np.arange(NVT)[None, None, :]).astype(f)  # [B,NV,NVT]
    in_maps = []
    for c in range(NCORES):
        s = slice(c * P, (c + 1) * P)
        xt = oh[s].transpose(2, 1, 0).reshape(NVT, NV * P)           # [16, NV*P]
        xt1 = np.concatenate([xt, np.ones((1, NV * P), f)], 0)
        xp = params[s].transpose(2, 1, 0).reshape(FS, NV * P).astype(f)
        xp1 = np.concatenate([xp, np.ones((1, NV * P), f)], 0)
        m = dict(shared)
        m["xt1"] = np.ascontiguousarray(xt1)
        m["xp1"] = np.ascontiguousarray(xp1)
        m["adjt"] = np.ascontiguousarray(adj[s].reshape(P, NV * NV)).astype(f)
        in_maps.append(m)
    return in_maps


_NC_CACHE = {}


def _get_nc():
    key = str(MMDT)
    if key not in _NC_CACHE:
        _NC_CACHE[key] = build_bass()
    return _NC_CACHE[key]


F32_INPUTS = {"adjt", "eye"}


def kernel(**inputs):
    np_inputs = {k: np.asarray(v) for k, v in inputs.items()}
    in_maps = _host_prep(**np_inputs)
    npdt = mybir.dt.np(MMDT)
    if npdt != np.float32:
        in_maps = [{k: (v if k in F32_INPUTS else v.astype(npdt))
                    for k, v in m.items()} for m in in_maps]
    nc = _get_nc()
    res = run_bass_kernel_spmd(nc, in_maps, core_ids=list(range(NCORES)),
                               **_RUN_KWARGS)
    out = np.concatenate([res.results[c]["out"] for c in range(NCORES)], 0)
    _LAST_RESULT.clear()
    _LAST_RESULT.append(res)
    return out[:, :NZ], out[:, NZ:]


# test.py can set these to enable tracing / inspect results
_RUN_KWARGS = {}
_LAST_RESULT = []
